# revision 1
# baseline (speedup 1.0000x reference)
"""Trainium2 Bass kernel for nn_ModAttn (modulated multi-function attention).

Shapes: x [1,1024,512], compatibility [1,4,1024]; out [1,4,1024,512].
Sharding: 8 cores = (function f in 0..3) x (n-half in 0..1). Each core
computes the full attention for its function over its 512 query rows
(k/v over all 1024 keys) and its [512, 512] slice of the output. No
collectives; host concatenates slices.

Per-core dataflow (one SPMD program, per-core differences only in data):
  cm        = layernorm((w_c @ code_f)^T)            [1, 512]   (both param sets)
  xmT       = x^T * cm_q (per-partition)             [512, 1024] fp32r
  q^T,k^T   = (W_qkv^T chunks)^T-matmul              fp32r, head-pairs stacked
  v         = natural layout + ones column           bf16 [128, 8*65] tiles
  C         = comp^T comp (outer, summed over f)     bf16
  per head h: S = q^T.T@k^T (K=64, fp32r) -> exp1(ACT, accum s) -> E1 bf16
              T = (E1 * 1/s) * C (one DVE op) -> PE-transpose -> T^T (bf16 PSUM)
              E2T = exp2(ACT reads PSUM) -> PV matmul (M=65: y^T rows + z2 row)
              ym^T = y^T * cm_p * (1/z2 broadcast)
  y         = ym^T.T @ W_proj^T + b_proj             fp32r
"""

import os
import numpy as np
from contextlib import ExitStack

PHASES = os.environ.get("BUILD_PHASES", "ABCDE")

N_CORES = 8
N, DIN, NF, H, HD = 1024, 512, 4, 8, 64
NHALF = 512
SCALE = HD ** -0.5

_CACHE = {}


def build_nc():
    import concourse.bacc as bacc
    import concourse.tile as tile
    from concourse import mybir
    from concourse.masks import make_identity

    F32 = mybir.dt.float32
    F32R = mybir.dt.float32r
    BF16 = mybir.dt.bfloat16
    AT = mybir.ActivationFunctionType
    OP = mybir.AluOpType

    nc = bacc.Bacc("TRN2", target_bir_lowering=False, debug=False,
                   num_devices=N_CORES)

    xT_d = nc.dram_tensor("xT", [DIN, N], F32, kind="ExternalInput")
    comp_d = nc.dram_tensor("comp", [NF, N], F32, kind="ExternalInput")
    codef_d = nc.dram_tensor("codef", [128, 1], F32, kind="ExternalInput")
    wct_d = nc.dram_tensor("wct", [128, DIN], F32, kind="ExternalInput")
    wqkvT_d = nc.dram_tensor("wqkvT", [DIN, 3 * DIN], F32R, kind="ExternalInput")
    wprojT_d = nc.dram_tensor("wprojT", [DIN, DIN], F32R, kind="ExternalInput")
    bqkv_d = nc.dram_tensor("bqkv", [3 * DIN], F32, kind="ExternalInput")
    bproj_d = nc.dram_tensor("bproj", [1, DIN], F32, kind="ExternalInput")
    lnqg_d = nc.dram_tensor("lnqg", [1, DIN], F32, kind="ExternalInput")
    lnqb_d = nc.dram_tensor("lnqb", [1, DIN], F32, kind="ExternalInput")
    lnpg_d = nc.dram_tensor("lnpg", [1, DIN], F32, kind="ExternalInput")
    lnpb_d = nc.dram_tensor("lnpb", [1, DIN], F32, kind="ExternalInput")
    y_d = nc.dram_tensor("y", [NHALF, DIN], F32, kind="ExternalOutput")

    with tile.TileContext(nc) as tc, ExitStack() as top:
        const = top.enter_context(tc.tile_pool(name="const", bufs=1))
        ident = const.tile([128, 128], BF16, tag="ident")
        make_identity(nc, ident[:])
        ones_r = const.tile([1, 128], F32, tag="ones_r")
        nc.vector.memset(ones_r[:], 1.0)
        ones_bf = const.tile([128, 1], BF16, tag="ones_bf")
        nc.vector.memset(ones_bf[:], 1.0)
        ones_rb = const.tile([1, 128], BF16, tag="ones_rb")
        nc.vector.memset(ones_rb[:], 1.0)
        cmT_q = const.tile([128, 4], F32, tag="cmT_q")
        cmT_p = const.tile([64, 8], F32, tag="cmT_p")  # column h = cm_p[h*64:(h+1)*64]
        # big input loads first so HBM transfers overlap the setup chain
        big = top.enter_context(tc.tile_pool(name="big", bufs=1))
        wq = [big.tile([128, 3 * DIN], F32R, tag=f"wq{c}", name=f"wq{c}") for c in range(4)]
        xt = [big.tile([128, N], F32, tag=f"xt{c}", name=f"xt{c}") for c in range(4)]
        wp = [big.tile([128, DIN], F32R, tag=f"wp{c}", name=f"wp{c}") for c in range(4)]
        for c in range(4):
            nc.gpsimd.dma_start(xt[c][:], xT_d.ap()[c * 128:(c + 1) * 128, :])
        for c in range(4):
            nc.gpsimd.dma_start(wq[c][:], wqkvT_d.ap()[c * 128:(c + 1) * 128, :])
        for c in range(4):
            nc.gpsimd.dma_start(wp[c][:], wprojT_d.ap()[c * 128:(c + 1) * 128, :])

        bp_raw = const.tile([1, DIN], F32, tag="bp_raw")
        nc.sync.dma_start(bp_raw[:], bproj_d.ap())
        bp_row = const.tile([1, DIN], BF16, tag="bp_row")
        nc.vector.tensor_copy(bp_row[:], bp_raw[:])

        # ---------- phase A: modulation vectors ----------
        with tc.tile_pool(name="smA", bufs=1) as smA, \
             tc.tile_pool(name="psA", bufs=1, space="PSUM") as psA:
            wct_t = smA.tile([128, DIN], F32, tag="wct")
            nc.sync.dma_start(wct_t[:], wct_d.ap())
            codef_t = smA.tile([128, 1], F32, tag="codef")
            nc.sync.dma_start(codef_t[:], codef_d.ap())
            lnt = {}
            for nm, d in (("qg", lnqg_d), ("qb", lnqb_d), ("pg", lnpg_d),
                          ("pb", lnpb_d)):
                lnt[nm] = smA.tile([1, DIN], F32, tag=f"ln{nm}", name=f"ln{nm}")
                nc.sync.dma_start(lnt[nm][:], d.ap())

            cm0_ps = psA.tile([1, DIN], F32, tag="cm0ps")
            nc.tensor.matmul(cm0_ps[:], codef_t[:], wct_t[:], start=True,
                             stop=True)
            cm0 = smA.tile([1, DIN], F32, tag="cm0")
            nc.vector.tensor_copy(cm0[:], cm0_ps[:])

            # shared LN stats for both param sets (same cm0)
            st = smA.tile([1, 1], F32, tag="st")
            nc.vector.tensor_reduce(st[:], cm0[:], mybir.AxisListType.X, OP.add)
            mu = smA.tile([1, 1], F32, tag="mu")
            nc.vector.tensor_scalar_mul(mu[:], st[:], 1.0 / DIN)
            sq = smA.tile([1, DIN], F32, tag="sq")
            vacc = smA.tile([1, 1], F32, tag="vacc")
            nc.vector.scalar_tensor_tensor(sq[:], cm0[:], mu[:], cm0[:],
                                           OP.subtract, OP.mult,
                                           accum_out=vacc[:])
            ve = smA.tile([1, 1], F32, tag="ve")
            nc.vector.tensor_scalar(ve[:], vacc[:], 1.0 / DIN, 1e-5,
                                    OP.mult, OP.add)
            sd = smA.tile([1, 1], F32, tag="sd")
            nc.scalar.activation(sd[:], ve[:], AT.Sqrt)
            rstd = smA.tile([1, 1], F32, tag="rstd")
            nc.vector.reciprocal(rstd[:], sd[:])

            def layer_norm(pref, g, b):
                rg = smA.tile([1, DIN], F32, tag=f"{pref}rg")
                nc.vector.tensor_scalar_mul(rg[:], g[:], rstd[:])
                cx = smA.tile([1, DIN], F32, tag=f"{pref}cx")
                nc.vector.scalar_tensor_tensor(cx[:], cm0[:], mu[:], rg[:],
                                               OP.subtract, OP.mult)
                cm = smA.tile([1, DIN], F32, tag=f"{pref}cm")
                nc.vector.tensor_add(cm[:], cx[:], b[:])
                return cm

            cmq = layer_norm("q", lnt["qg"], lnt["qb"])
            cmp_ = layer_norm("p", lnt["pg"], lnt["pb"])
            for c in range(4):
                tp = psA.tile([128, 1], F32, tag="cmtp", name="cmtp")
                nc.tensor.transpose(tp[:], cmq[:, c * 128:(c + 1) * 128],
                                    ones_r[0:1, 0:1])
                nc.vector.tensor_copy(cmT_q[:, c:c + 1], tp[:])
            for h in range(8):
                tp = psA.tile([128, 1], F32, tag="cmtp", name="cmtp")
                nc.tensor.transpose(tp[0:64, :], cmp_[:, h * 64:(h + 1) * 64],
                                    ones_r[0:1, 0:1])
                nc.vector.tensor_copy(cmT_p[:, h:h + 1], tp[0:64, :])

        # ---------- persistent attention operands ----------
        qkv = top.enter_context(tc.tile_pool(name="qkv", bufs=1))
        qT = [qkv.tile([128, NHALF], F32R, tag=f"qT{j}", name=f"qT{j}") for j in range(4)]
        kT = [qkv.tile([128, N], F32R, tag=f"kT{j}", name=f"kT{j}") for j in range(4)]
        vv = [qkv.tile([128, H * (HD + 1)], BF16, tag=f"vv{m}", name=f"vv{m}")
              for m in range(8)]
        Ct = [qkv.tile([128, N], BF16, tag=f"C{m}", name=f"C{m}") for m in range(4)]
        ymT = [qkv.tile([128, NHALF], F32R, tag=f"ymT{c}", name=f"ymT{c}") for c in range(4)]

        # ---------- phase C: compatibility outer product ----------
        if "C" in PHASES:
         with tc.tile_pool(name="smC", bufs=1) as smC, \
             tc.tile_pool(name="psC", bufs=2, space="PSUM") as psC:
            comp_raw = smC.tile([NF, N], F32, tag="comp_raw")
            nc.sync.dma_start(comp_raw[:], comp_d.ap())
            comp_r = smC.tile([NF, N], F32R, tag="comp_r")
            nc.vector.tensor_copy(comp_r[:], comp_raw[:])
            for mc2 in range(4):
                ps = psC.tile([128, N], F32, tag="psc", name="psc")
                for half in range(2):
                    mc = 2 * mc2 + half
                    nc.tensor.matmul(ps[:, half * 512:(half + 1) * 512],
                                     comp_r[:, mc * 128:(mc + 1) * 128],
                                     comp_r[:, 0:NHALF], start=True, stop=True)
                nc.vector.tensor_copy(Ct[mc2][:], ps[:])

        # ---------- phase B: QKV projections ----------
        if "B" not in PHASES:
            for t in qT + kT + vv + Ct + ymT:
                ap = t[:]
                if ap.dtype == F32R:
                    ap = ap.bitcast(F32)
                nc.vector.memset(ap, 0.0)
        elif True:
         with tc.tile_pool(name="smB", bufs=1) as smB, \
             tc.tile_pool(name="psB", bufs=3, space="PSUM") as psB, \
             tc.tile_pool(name="psK", bufs=2, space="PSUM") as psK:
            xm = [smB.tile([128, N], F32R, tag=f"xm{c}", name=f"xm{c}") for c in range(4)]
            bqk_t = smB.tile([128, 8], F32, tag="bqk")
            for c in range(4):
                nc.vector.tensor_scalar_mul(xm[c][:], xt[c][:], cmT_q[:, c:c + 1])
            for j in range(8):
                nc.sync.dma_start(bqk_t[:, j:j + 1],
                                  bqkv_d.ap()[j * 128:(j + 1) * 128])
            bv_raw = smB.tile([1, DIN], F32, tag="bv_raw")
            nc.sync.dma_start(bv_raw[:], bqkv_d.ap()[2 * DIN:3 * DIN])
            bv_row = smB.tile([1, DIN], BF16, tag="bv_row")
            nc.vector.tensor_copy(bv_row[:], bv_raw[:])

            for j in range(4):  # q^T: head-pair tiles [128, 512]
                ps = psB.tile([128, NHALF], F32, tag="psb", name="psb")
                for c in range(4):
                    nc.tensor.matmul(ps[:], wq[c][:, j * 128:(j + 1) * 128],
                                     xm[c][:, 0:NHALF], start=(c == 0),
                                     stop=(c == 3))
                nc.vector.tensor_scalar_add(qT[j][:], ps[:], bqk_t[:, j:j + 1])
            for j in range(4):  # k^T: head-pair tiles [128, 1024]
                ps = psK.tile([128, N], F32, tag="psk")
                for half in range(2):
                    for c in range(4):
                        nc.tensor.matmul(
                            ps[:, half * 512:(half + 1) * 512],
                            wq[c][:, DIN + j * 128:DIN + (j + 1) * 128],
                            xm[c][:, half * 512:(half + 1) * 512],
                            start=(c == 0), stop=(c == 3))
                nc.vector.tensor_scalar_add(kT[j][:], ps[:],
                                            bqk_t[:, 4 + j:5 + j])
            for m in range(8):  # v natural [128 rows of m, 512] + ones cols
                ps = psB.tile([128, DIN], F32, tag="psb", name="psb")
                for c in range(4):
                    nc.tensor.matmul(ps[:], xm[c][:, m * 128:(m + 1) * 128],
                                     wq[c][:, 2 * DIN:3 * DIN],
                                     start=(c == 0), stop=False)
                nc.tensor.matmul(ps[:], ones_rb[:], bv_row[:], start=False,
                                 stop=True)
                v3 = vv[m][:].rearrange("p (h e) -> p h e", e=HD + 1)
                nc.vector.tensor_copy(v3[:, :, 0:HD],
                                      ps[:].rearrange("p (h e) -> p h e", e=HD))
                nc.vector.memset(v3[:, :, HD:HD + 1], 1.0)

        # ---------- phase D: attention, transposed orientation ----------
        # Software-pipelined: D1(h) = scores/exp1/s-reduce, D2(h) = T/exp2/PV.
        # Emission order D1(0), D1(1), D2(0), D1(2), D2(1), ... keeps PE fed
        # with independent matmul work (HAM stays warm).
        if "D" in PHASES:
         with tc.tile_pool(name="smE1", bufs=12) as smE1, \
             tc.tile_pool(name="smD", bufs=3) as smD, \
             tc.tile_pool(name="smZ", bufs=2) as smZ, \
             tc.tile_pool(name="psS", bufs=3, space="PSUM") as psS, \
             tc.tile_pool(name="psZ", bufs=1, space="PSUM") as psZ, \
             tc.tile_pool(name="psY", bufs=1, space="PSUM") as psY:
            state = {}

            def emit_d1(h):
                hp, ho = h // 2, (h % 2) * 64
                s_ps = psZ.tile([1, NHALF], F32, tag="s_ps", name="s_ps")
                e1_tiles = []
                for mc2 in range(4):
                    ps = psS.tile([128, N], F32, tag="ps_s", name="ps_s")
                    e1 = smE1.tile([128, N], BF16, tag="e1", name="e1")
                    for half in range(2):
                        mc = 2 * mc2 + half
                        nc.tensor.matmul(
                            ps[:, half * 512:(half + 1) * 512],
                            kT[hp][ho:ho + 64, mc * 128:(mc + 1) * 128],
                            qT[hp][ho:ho + 64, :], start=True, stop=True)
                    nc.scalar.activation(e1[:], ps[:], AT.Exp, scale=SCALE)
                    for half in range(2):
                        mc = 2 * mc2 + half
                        nc.tensor.matmul(
                            s_ps[:], ones_bf[:],
                            e1[:, half * 512:(half + 1) * 512],
                            start=(mc == 0), stop=(mc == 7))
                    e1_tiles.append(e1)
                srow = smZ.tile([1, NHALF], F32, tag="srow", name="srow")
                nc.scalar.copy(srow[:], s_ps[:])
                sraw = smZ.tile([128, NHALF], F32, tag="sraw", name="sraw")
                nc.gpsimd.partition_broadcast(sraw[:], srow[:], channels=128)
                rsf = smZ.tile([128, NHALF], F32, tag="rsf", name="rsf")
                nc.vector.reciprocal_approx_fast(rsf[:], sraw[:])
                rsb = smZ.tile([128, N], BF16, tag="rsb", name="rsb")
                nc.vector.tensor_copy(rsb[:, 0:NHALF], rsf[:])
                nc.scalar.copy(rsb[:, NHALF:N], rsf[:])
                state[h] = (e1_tiles, rsb)

            def emit_d2(h):
                hp, ho = h // 2, (h % 2) * 64
                e1_tiles, rsb = state.pop(h)
                ypv = psY.tile([HD + 1, NHALF], F32, tag="ypv", name="ypv")
                for q4 in range(2):
                    t2 = smD.tile([128, 2 * N], BF16, tag="t2", name="t2")
                    for sub in range(2):
                        mc2 = 2 * q4 + sub
                        t1 = smD.tile([128, N], BF16, tag="t1", name="t1")
                        nc.vector.tensor_mul(t1[:], e1_tiles[mc2][:], rsb[:])
                        nc.vector.tensor_mul(t2[:, sub * N:(sub + 1) * N],
                                             t1[:], Ct[mc2][:])
                    e2 = smD.tile([128, 2 * N], BF16, tag="e2", name="e2")
                    nc.scalar.activation(e2[:], t2[:], AT.Exp)
                    for qq in range(4):
                        mc = 4 * q4 + qq
                        nc.tensor.matmul(
                            ypv[:],
                            vv[mc][:, h * (HD + 1):(h + 1) * (HD + 1)],
                            e2[:, qq * 512:(qq + 1) * 512],
                            start=(mc == 0), stop=(mc == 7))
                yraw = smZ.tile([HD + 1, NHALF], F32, tag="yraw", name="yraw")
                nc.scalar.copy(yraw[:], ypv[:])
                zr = smZ.tile([1, NHALF], F32, tag="zr", name="zr")
                nc.vector.tensor_copy(zr[:], yraw[HD:HD + 1, :])
                zb = smZ.tile([64, NHALF], F32, tag="zb", name="zb")
                nc.gpsimd.partition_broadcast(zb[:], zr[:], channels=64)
                rz = smZ.tile([64, NHALF], F32, tag="rz", name="rz")
                nc.vector.reciprocal_approx_fast(rz[:], zb[:])
                nc.vector.scalar_tensor_tensor(
                    ymT[hp][ho:ho + 64, :], yraw[0:HD, :],
                    cmT_p[:, h:h + 1], rz[:], OP.mult, OP.mult)

            emit_d1(0)
            emit_d1(1)
            for h in range(2, H):
                emit_d1(h)
                emit_d2(h - 2)
            emit_d2(H - 2)
            emit_d2(H - 1)

        # ---------- phase E: output projection ----------
        if "E" not in PHASES:
            with tc.tile_pool(name="smE0", bufs=1) as smE0:
                yo0 = smE0.tile([128, DIN], F32, tag="yo0")
                nc.vector.memset(yo0[:], 0.0)
                for nb in range(4):
                    nc.sync.dma_start(y_d.ap()[nb * 128:(nb + 1) * 128, :], yo0[:])
        elif True:
         with tc.tile_pool(name="smE", bufs=2) as smE, \
             tc.tile_pool(name="psE", bufs=2, space="PSUM") as psE:
            for nb in range(4):
                ps = psE.tile([128, DIN], F32, tag="ps_e")
                for c in range(4):
                    nc.tensor.matmul(ps[:], ymT[c][:, nb * 128:(nb + 1) * 128],
                                     wp[c][:], start=(c == 0), stop=False)
                nc.tensor.matmul(ps[:], ones_rb[:], bp_row[:], start=False,
                                 stop=True)
                yo = smE.tile([128, DIN], F32, tag="yo")
                nc.vector.tensor_copy(yo[:], ps[:])
                nc.sync.dma_start(y_d.ap()[nb * 128:(nb + 1) * 128, :], yo[:])

    nc.compile()
    return nc


def make_in_maps(x, compatibility, code, w_c, W_qkv, b_qkv, W_proj, b_proj,
                 ln_qkv_g, ln_qkv_b, ln_proj_g, ln_proj_b):
    x = np.asarray(x, np.float32)
    compatibility = np.asarray(compatibility, np.float32)
    shared = {
        "wct": np.ascontiguousarray(np.asarray(w_c, np.float32).T),
        "wqkvT": np.ascontiguousarray(np.asarray(W_qkv, np.float32).T),
        "wprojT": np.ascontiguousarray(np.asarray(W_proj, np.float32).T),
        "bqkv": np.asarray(b_qkv, np.float32),
        "bproj": np.asarray(b_proj, np.float32).reshape(1, DIN),
        "lnqg": np.asarray(ln_qkv_g, np.float32).reshape(1, DIN),
        "lnqb": np.asarray(ln_qkv_b, np.float32).reshape(1, DIN),
        "lnpg": np.asarray(ln_proj_g, np.float32).reshape(1, DIN),
        "lnpb": np.asarray(ln_proj_b, np.float32).reshape(1, DIN),
    }
    code = np.asarray(code, np.float32)
    xT = np.ascontiguousarray(x[0].T)  # [512, 1024]
    cp = compatibility[0]  # [4, 1024]
    in_maps = []
    for core in range(N_CORES):
        f, half = core // 2, core % 2
        idx = np.r_[half * NHALF:(half + 1) * NHALF,
                    (1 - half) * NHALF:(2 - half) * NHALF]
        in_maps.append(dict(
            shared,
            xT=np.ascontiguousarray(xT[:, idx]),
            comp=np.ascontiguousarray(cp[:, idx]),
            codef=np.ascontiguousarray(code[:, f:f + 1]),
        ))
    return in_maps


def kernel(**inputs) -> np.ndarray:
    from concourse.bass_utils import run_bass_kernel_spmd
    if "nc" not in _CACHE:
        _CACHE["nc"] = build_nc()
    nc = _CACHE["nc"]
    in_maps = make_in_maps(**inputs)
    res = run_bass_kernel_spmd(nc, in_maps, core_ids=list(range(N_CORES)))
    out = np.empty((1, NF, N, DIN), np.float32)
    for core in range(N_CORES):
        f, half = core // 2, core % 2
        out[0, f, half * NHALF:(half + 1) * NHALF, :] = res.results[core]["y"]
    return out

